# revision 18
# baseline (speedup 1.0000x reference)
"""BernNet (nn_BernNet_86492051407432) Trainium2 kernel — 8 NeuronCores.

Math: reference computes out = log_softmax(P(A) h) where
h = relu(x@W1+b1)@W2+b2 and P is the Bernstein polynomial
  P = (1/2^K) sum_k C(K,k) TEMP[k] (I-A)^k (I+A)^{K-k}.
Expanding in monomials of A: P = sum_j c_j A^j with coefficients c_j
computable exactly on the host from TEMP.  For TEMP = relu(ones) = ones
(what setup_inputs produces), the binomial sum telescopes:
  sum_k C(K,k) (I-A)^k (I+A)^{K-k} = ((I-A)+(I+A))^K = 2^K I
so c_0 = 1, c_j = 0 (j>=1) EXACTLY (integer arithmetic), and the output
is log_softmax(h) with no graph propagation at all.

The device kernel computes the MLP logits po = relu(x@W1+b1)@W2 + b2,
row-sharded across the 8 cores (embarrassingly parallel).  The softmax
epilogue (subtract logsumexp) runs on the host from the logits the
device already returns.  A host fallback handles the general-temp case
(never hit by setup_inputs).

Performance design (v4):
 - x ships as fp8 e3m4: |x|max ~ 5.4 fits +-15.5 and the quantization
   noise lands at rel-err ~8e-3 vs the 2e-2 gate (measured end-to-end
   on the real inputs).  Halves the dominant HBM traffic vs fp16.
 - Chunks use 128 partitions (features padded 500->512).  The HWDGE
   splits a DMA across engines by factoring the outer AP dim: 128 ->
   [16 x 8] uses all 16 SDMA engines (~340 GB/s); 125 -> [5 x 25] got
   only 5 engines (~125 GB/s).  Weights go on the same fast sync/HWDGE
   queue BEFORE the x stream (the gpsimd/SWDGE path delivered them
   ~15 us late and stalled the first matmuls).
 - PE column tiling, everything in the (128,64) tile config: array
   columns 0-63 (lane A) and 64-127 (lane B) run concurrently with
   independent weights and moving streams.  Work is organized in
   QUADS of four 512-row groups: lane A computes groups 4q,4q+1 while
   lane B computes 4q+2,4q+3.  mm1+mm2 for a quad is 20 interleaved
   N=512 streams over 2 lanes -> ~2.1 us warm, under the quad's DMA
   time (~3 us) -> the steady state is input-DMA-bound.
 - ph/po PSUM tiles hold a whole quad ([128,1024] = 2 banks), so each
   elementwise instruction covers TWO groups (1024 cols), halving the
   per-instruction overhead that throttled v3: per quad just 4 ops,
   balanced 2+2 over ACT and DVE (relu-A + cast-B on ACT, relu-B via
   tensor_scalar + cast-A on DVE).
 - hT is 3 persistent [65,1024] lane buffers whose row 64 is a
   constant 1 (written once) so W2's extra row folds b2 into mm2.
 - Input DMA: progressive regions; output: chunked flushes on the
   scalar HWDGE queue overlapping compute.  25 groups total (12800
   rows, only 300 rows of padding); the last "quad" has one group.
"""

import math

import numpy as np

N, E = 100000, 1600000
F_IN, HID, CLS, K = 500, 64, 64, 10
KC = 128                     # contraction chunk partitions (features padded 500->512)
HIDP = HID + 1               # extra constant-1 hidden row carries b2
N_CORES = 8
RPC = N // N_CORES           # real rows per core: 12500
GROUP = 512                  # rows per PSUM matmul group
NG_TOT = 25                  # groups per core (6 quads + 1 lone group)
RPAD = NG_TOT * GROUP        # padded rows per core: 12800
NQ = (NG_TOT + 3) // 4       # quad iterations (last may be partial)
N_HT = 6                     # hT lane-buffer ring (3 quads of WAR slack)
OUT_FLUSH = [2048, 4096, 6144, 8192, 10240, 12288, 12800]
# tapered input-DMA regions (rows), all on the sync HWDGE queue: a
# matmul waits on its whole region's completion, descriptors must stay
# >= 4KB for stream rate, and the last regions are small (their
# completion latency gates the tail).  Splitting regions across both
# HWDGE queues was tried and is WORSE: fair packet round-robin makes
# every region complete later.
XT_REGIONS = [(0, 1024), (1024, 3072), (4096, 3072), (7168, 3072),
              (10240, 2560)]
NAT_LOG_EXP_SET = 6          # act_info.json id of natural_log_exp_and_others

_CACHE: dict = {}


def _bernstein_monomial_coeffs(temp: np.ndarray) -> np.ndarray:
    """Exact monomial coefficients c_j of
    (1/2^K) sum_i C(K,i) TEMP[i] (I-A)^i (I+A)^{K-i}  in powers of A.

    Uses float64 on small integers (exactly representable), so for
    TEMP == 1 the j>=1 coefficients cancel to exactly 0.0.
    """
    TEMP = np.maximum(temp.astype(np.float64), 0.0)
    c = np.zeros(K + 1, dtype=np.float64)
    for i in range(K + 1):
        p1 = np.array([math.comb(i, j) * ((-1.0) ** j) for j in range(i + 1)])
        p2 = np.array([math.comb(K - i, j) * 1.0 for j in range(K - i + 1)])
        c += math.comb(K, i) * TEMP[i] * np.convolve(p1, p2)
    return c / (2.0 ** K)


def _host_reference(x, edge_index, W1, b1, W2, b2, temp):
    """Full-fidelity host fallback (general temp).  Never hit for the
    setup_inputs() distribution (temp == ones); kept for correctness."""
    h = np.maximum(x @ W1 + b1, 0.0) @ W2 + b2
    row, col = edge_index[0].astype(np.int64), edge_index[1].astype(np.int64)
    deg = np.bincount(row, minlength=N).astype(np.float32)
    dis = np.where(deg > 0, 1.0 / np.sqrt(np.where(deg > 0, deg, 1.0)), 0.0)
    w = (dis[row] * dis[col]).astype(np.float32)
    try:
        import scipy.sparse as sp

        A = sp.csr_matrix((w, (row, col)), shape=(N, N), dtype=np.float32)

        def Av(v):
            return A @ v
    except ImportError:
        order = np.argsort(row, kind="stable")
        rs, cs, ws = row[order], col[order], w[order]
        starts = np.searchsorted(rs, np.arange(N))

        def Av(v):
            contrib = ws[:, None] * v[cs]
            out = np.add.reduceat(
                np.concatenate([contrib, np.zeros((1, v.shape[1]), v.dtype)]),
                np.minimum(starts, len(rs)),
                axis=0,
            )[:N]
            out[np.diff(np.append(starts, len(rs))) == 0] = 0
            return out

    TEMP = np.maximum(temp, 0.0)
    tmp = [h]
    v = h
    for _ in range(K):
        v = v + Av(v)
        tmp.append(v)
    out = (math.comb(K, 0) / 2 ** K) * TEMP[0] * tmp[K]
    for i in range(K):
        v = tmp[K - i - 1]
        for _ in range(i + 1):
            v = v - Av(v)
        out = out + (math.comb(K, i + 1) / 2 ** K) * TEMP[i + 1] * v
    m = out.max(axis=1, keepdims=True)
    return (out - m - np.log(np.exp(out - m).sum(axis=1, keepdims=True))).astype(
        np.float32
    )


def _dedupe_act_table_loads(nc, mybir):
    """Rewrite every act-table load to NAT_LOG_EXP_SET (covers Relu,
    Identity) and drop all but the first load per block."""
    for blk in nc.main_func.blocks:
        seen = False
        keep = []
        for inst in blk.instructions:
            if isinstance(inst, mybir.InstLoadActFuncSet):
                inst.act_func_set_id = NAT_LOG_EXP_SET
                plain = (
                    not inst.sync_info
                    and not inst.has_wait()
                    and not inst.has_update()
                )
                if seen and plain:
                    continue
                seen = True
            keep.append(inst)
        if len(keep) != len(blk.instructions):
            del blk.instructions[:]
            for inst in keep:
                blk.instructions.append(inst)


def _build_nc():
    """Build + compile the per-core Bass module (cached)."""
    if "nc" in _CACHE:
        return _CACHE["nc"]

    import concourse.bass as bass
    import concourse.tile as tile
    from concourse import bacc, mybir

    f32 = mybir.dt.float32
    f16 = mybir.dt.float16
    f8 = mybir.dt.float8e3
    AF = mybir.ActivationFunctionType
    ALU = mybir.AluOpType

    nc = bacc.Bacc("TRN2", target_bir_lowering=False, debug=False)

    xt = nc.declare_dram_parameter("xt", [KC, 4 * RPAD], f8, isOutput=False)
    w1 = nc.declare_dram_parameter("w1", [KC, 4 * HID], f16, isOutput=False)
    b1 = nc.declare_dram_parameter("b1", [HID, 1], f32, isOutput=False)
    w2 = nc.declare_dram_parameter("w2", [HIDP, CLS], f16, isOutput=False)
    out = nc.declare_dram_parameter("out", [CLS, RPAD], f16, isOutput=True)

    def groups_of(q):
        return [g for g in range(4 * q, 4 * q + 4) if g < NG_TOT]

    with tile.TileContext(nc) as tc:
        with (
            tc.tile_pool(name="const", bufs=1) as constp,
            tc.tile_pool(name="outp", bufs=3) as op,
            tc.tile_pool(name="psum", bufs=2, space=bass.MemorySpace.PSUM) as pp,
            tc.tile_pool(name="psum3", bufs=2, space=bass.MemorySpace.PSUM) as pp3,
        ):
            # --- weights on the scalar HWDGE queue: concurrent with the
            # x stream on the sync queue (both rings are idle at start).
            # w1 is pre-baked host-side to one contiguous 512 B run per
            # partition (the naive rearrange was 512 x 128 B descriptors
            # and stalled the queue ~5 us).
            w1_sb = constp.tile([KC, 4, HID], f16)
            nc.scalar.dma_start(out=w1_sb[:], in_=w1[:])
            w2_sb = constp.tile([HIDP, CLS], f16)
            nc.scalar.dma_start(out=w2_sb[:], in_=w2[:])
            b1_sb = constp.tile([HID, 1], f32)
            nc.scalar.dma_start(out=b1_sb[:], in_=b1[:])

            # persistent hT lane buffers (2 groups each); row 64 is the
            # constant 1 that turns W2's b2 row into a bias add in mm2.
            ht_bufs = []
            for i in range(N_HT):
                hb = constp.tile([HIDP, 2 * GROUP], f16, name=f"htb{i}")
                ht_bufs.append(hb)

            # Warmup/pacing bridge.  The x stream supplies ~360 GB/s but
            # the column-tiled PE consumes ~490, so a PE that starts as
            # soon as region 0 lands keeps catching up to the stream in
            # 1-2 us bursts that re-throttle the HAM clock gate (half
            # clock) and pay the region-completion latency repeatedly.
            # Instead, concentrate ALL the deficit at the start: batches
            # of throwaway matmuls gated on successive regions keep the
            # PE continuously busy until ~3 regions are resident, after
            # which the real stream runs warm and uninterrupted to the
            # end.  The throwaway results land in ph0 windows that the
            # first real start=True matmuls overwrite.
            warm_in = constp.tile([KC, GROUP], f16)
            nc.vector.memset(warm_in[:], 0.0)
            for hb in ht_bufs:
                nc.vector.memset(hb[HID : HID + 1, :], 1.0)

            xt_all = constp.tile([KC, 4 * RPAD], f8)
            reg_of = {}
            for ri, (s, rows) in enumerate(XT_REGIONS):
                nc.sync.dma_start(
                    out=xt_all[:, 4 * s : 4 * s + 4 * rows],
                    in_=xt[:, 4 * s : 4 * s + 4 * rows],
                )
                for g in range(s // GROUP, (s + rows) // GROUP):
                    reg_of[g] = (s, rows)

            def xt_ap(g, ci):
                s, rows = reg_of[g]
                a = g * GROUP - s
                off = 4 * s + ci * rows + a
                return xt_all[:, off : off + GROUP]

            # Warmup: 8 throwaway matmuls rotating over ph0's four
            # windows (same-window WAW back-to-back would serialize at
            # full fill+drain latency) bridge the PE from t~8 until
            # region 0 lands; the first real start=True matmuls then
            # overwrite ph0.
            ph0 = pp3.tile([KC, 2 * GROUP], f32, tag="ph")
            phs = {0: ph0}
            for wi in range(8):
                rp = 64 * (wi % 2)
                cw = GROUP * ((wi // 2) % 2)
                nc.tensor.matmul(
                    ph0[rp : rp + 64, cw : cw + GROUP],
                    lhsT=warm_in[:, 0:64], rhs=warm_in[:],
                    start=True, stop=True, tile_position=(0, rp),
                )

            def win(tile_, g):
                """[64,512] window of a quad PSUM tile for group g."""
                rp = 64 * ((g % 4) // 2)
                cw = GROUP * (g % 2)
                return tile_[rp : rp + 64, cw : cw + GROUP]

            pos = {}
            ots = {}

            def s0_mm1(q):
                """mm1 for the quad.  Early quads (the DMA-deficit
                window) issue lane-major/serial so the PE consumes at
                ~245 GB/s and trails the ~360 GB/s stream via its
                natural data deps, staying continuously busy (warm HAM,
                no catch-up bursts).  Later quads interleave lane A and
                lane B so both column tiles stream concurrently at 2x.
                """
                gs = groups_of(q)
                if q in phs:
                    ph = phs[q]
                else:
                    ph = pp3.tile([KC, 2 * GROUP], f32, tag="ph")
                    phs[q] = ph
                laneA = [g for g in gs if (g % 4) < 2]
                laneB = [g for g in gs if (g % 4) >= 2]
                order = []
                for ci in range(4):
                    for sub in range(max(len(laneA), len(laneB))):
                        for lane in (laneA, laneB):
                            if sub < len(lane):
                                order.append((lane[sub], ci))
                for g, ci in order:
                    cp = 64 * ((g % 4) // 2)
                    nc.tensor.matmul(
                        win(ph, g),
                        lhsT=w1_sb[:, ci, :], rhs=xt_ap(g, ci),
                        start=(ci == 0), stop=(ci == 3),
                        tile_position=(0, cp),
                    )

            def s1_mid(q):
                gs = groups_of(q)
                ph = phs.pop(q)
                nA = len([g for g in gs if (g % 4) < 2])
                nB = len(gs) - nA
                hbA = ht_bufs[(2 * q) % N_HT]
                hbB = ht_bufs[(2 * q + 1) % N_HT]
                # relu lane A on ACT, lane B on DVE (tensor_scalar)
                if nA:
                    nc.scalar.activation(
                        hbA[0:HID, 0 : nA * GROUP], ph[0:64, 0 : nA * GROUP],
                        AF.Relu, bias=b1_sb[:],
                    )
                if nB:
                    nc.vector.tensor_scalar(
                        hbB[0:HID, 0 : nB * GROUP], ph[64:128, 0 : nB * GROUP],
                        scalar1=b1_sb[:], scalar2=0.0,
                        op0=ALU.add, op1=ALU.max,
                    )
                po = pp.tile([KC, 2 * GROUP], f32, tag="po")
                pos[q] = po
                for sub in range(2):
                    for g in gs:
                        if g % 2 != sub:
                            continue
                        cp = 64 * ((g % 4) // 2)
                        hb = hbA if (g % 4) < 2 else hbB
                        cw = GROUP * (g % 2)
                        nc.tensor.matmul(
                            win(po, g), lhsT=w2_sb[:],
                            rhs=hb[:, cw : cw + GROUP],
                            start=True, stop=True, tile_position=(0, cp),
                        )

            def s2_fin(q):
                gs = groups_of(q)
                po = pos.pop(q)
                g_end = (gs[-1] + 1) * GROUP
                bnd_prev = 0
                for b in OUT_FLUSH:
                    if b >= g_end:
                        bnd = b
                        break
                    bnd_prev = b
                t = bnd_prev
                if t not in ots:
                    ot_new = op.tile([CLS, bnd - bnd_prev], f16, tag="ot")
                    ots[t] = ot_new
                nA = len([g for g in gs if (g % 4) < 2])
                nB = len(gs) - nA
                c0 = gs[0] * GROUP - t
                # cast lane A on DVE, lane B on ACT
                if nA:
                    nc.vector.tensor_copy(
                        ots[t][:, c0 : c0 + nA * GROUP], po[0:64, 0 : nA * GROUP]
                    )
                if nB:
                    nc.scalar.activation(
                        ots[t][:, c0 + nA * GROUP : c0 + (nA + nB) * GROUP],
                        po[64:128, 0 : nB * GROUP], AF.Copy,
                    )
                if g_end == bnd:
                    # flush via the idle gpsimd SWDGE queue: on the
                    # scalar queue the waiting DMA head-of-line blocks
                    # the next quad's ReLU; on the sync queue it would
                    # serialize behind the remaining x stream.  The
                    # LAST flush goes on scalar (no ACTIVATEs remain,
                    # HWDGE trigger is ~1 us faster than SWDGE).
                    eng = nc.scalar if bnd == OUT_FLUSH[-1] else nc.gpsimd
                    eng.dma_start(out=out[:, t:bnd], in_=ots.pop(t)[:])

            for qq in range(NQ + 2):
                if qq < NQ:
                    s0_mm1(qq)
                if 1 <= qq <= NQ:
                    s1_mid(qq - 1)
                if 2 <= qq <= NQ + 1:
                    s2_fin(qq - 2)

    nc.compile()
    _dedupe_act_table_loads(nc, mybir)
    _CACHE["nc"] = nc
    return nc


def _bake_xt(x_rows: np.ndarray) -> np.ndarray:
    """[<=RPC, F_IN] row slice -> flat SBUF image [KC, 4*RPAD] fp8e3m4,
    features padded to 512 in 4 chunks of 128, laid out per DMA region
    as [partition][chunk][row-in-region]."""
    import ml_dtypes

    xp_ = np.zeros((4 * KC, RPAD), ml_dtypes.float8_e3m4)
    xp_[:F_IN, : x_rows.shape[0]] = x_rows.T.astype(ml_dtypes.float8_e3m4)
    parts = []
    for (s, rows) in XT_REGIONS:
        blk = xp_[:, s : s + rows].reshape(4, KC, rows).transpose(1, 0, 2)
        parts.append(blk.reshape(KC, 4 * rows))
    return np.ascontiguousarray(np.concatenate(parts, axis=1))


def kernel(**inputs: np.ndarray) -> np.ndarray:
    x = np.asarray(inputs["x"], dtype=np.float32)
    W1 = np.asarray(inputs["W1"], dtype=np.float32)
    b1 = np.asarray(inputs["b1"], dtype=np.float32)
    W2 = np.asarray(inputs["W2"], dtype=np.float32)
    b2 = np.asarray(inputs["b2"], dtype=np.float32)
    temp = np.asarray(inputs["temp"], dtype=np.float32)

    c = _bernstein_monomial_coeffs(temp)
    if np.any(c[1:] != 0.0) or c[0] != 1.0:
        # General temp: graph propagation actually matters — host fallback.
        return _host_reference(
            x, np.asarray(inputs["edge_index"]), W1, b1, W2, b2, temp
        )

    from concourse.bass_utils import run_bass_kernel_spmd

    nc = _build_nc()
    W1f = np.zeros((4 * KC, HID), np.float16)
    W1f[:F_IN, :] = W1.astype(np.float16)
    # [KC, 4*HID]: chunk-major per partition, matching w1_sb [KC,4,HID]
    W1p = np.ascontiguousarray(
        W1f.reshape(4, KC, HID).transpose(1, 0, 2).reshape(KC, 4 * HID)
    )
    b1p = np.ascontiguousarray(b1.reshape(HID, 1))
    # W2' = [W2 ; b2]: the constant-1 hT row turns row 64 into + b2
    W2p = np.ascontiguousarray(
        np.concatenate([W2, b2.reshape(1, CLS)], axis=0)
    ).astype(np.float16)

    in_maps = []
    for cix in range(N_CORES):
        in_maps.append(
            {
                "xt": _bake_xt(x[cix * RPC : (cix + 1) * RPC]),
                "w1": W1p,
                "b1": b1p,
                "w2": W2p,
            }
        )

    res = run_bass_kernel_spmd(nc, in_maps, list(range(N_CORES)))
    po = np.empty((N, CLS), np.float32)
    for cix in range(N_CORES):
        o = res.results[cix]["out"]
        po[cix * RPC : (cix + 1) * RPC] = o[:, :RPC].T
    # softmax epilogue (cheap, bandwidth-free on device): log_softmax(po)
    m = po.max(axis=1, keepdims=True)
    po -= m + np.log(np.exp(po - m).sum(axis=1, keepdims=True))
    return po


# revision 19
# speedup vs baseline: 1.2365x; 1.2365x over previous
"""BernNet (nn_BernNet_86492051407432) Trainium2 kernel — 8 NeuronCores.

Math: reference computes out = log_softmax(P(A) h) where
h = relu(x@W1+b1)@W2+b2 and P is the Bernstein polynomial
  P = (1/2^K) sum_k C(K,k) TEMP[k] (I-A)^k (I+A)^{K-k}.
Expanding in monomials of A: P = sum_j c_j A^j with coefficients c_j
computable exactly on the host from TEMP.  For TEMP = relu(ones) = ones
(what setup_inputs produces), the binomial sum telescopes:
  sum_k C(K,k) (I-A)^k (I+A)^{K-k} = ((I-A)+(I+A))^K = 2^K I
so c_0 = 1, c_j = 0 (j>=1) EXACTLY (integer arithmetic), and the output
is log_softmax(h) with no graph propagation at all.

The device kernel computes the MLP logits po = relu(x@W1+b1)@W2 + b2,
row-sharded across the 8 cores (embarrassingly parallel).  The softmax
epilogue (subtract logsumexp) runs on the host from the logits the
device already returns.  A host fallback handles the general-temp case
(never hit by setup_inputs).

Performance design (v4):
 - x ships as fp8 e3m4: |x|max ~ 5.4 fits +-15.5 and the quantization
   noise lands at rel-err ~8e-3 vs the 2e-2 gate (measured end-to-end
   on the real inputs).  Halves the dominant HBM traffic vs fp16.
 - Chunks use 128 partitions (features padded 500->512).  The HWDGE
   splits a DMA across engines by factoring the outer AP dim: 128 ->
   [16 x 8] uses all 16 SDMA engines (~340 GB/s); 125 -> [5 x 25] got
   only 5 engines (~125 GB/s).  Weights go on the same fast sync/HWDGE
   queue BEFORE the x stream (the gpsimd/SWDGE path delivered them
   ~15 us late and stalled the first matmuls).
 - PE column tiling, everything in the (128,64) tile config: array
   columns 0-63 (lane A) and 64-127 (lane B) run concurrently with
   independent weights and moving streams.  Work is organized in
   QUADS of four 512-row groups: lane A computes groups 4q,4q+1 while
   lane B computes 4q+2,4q+3.  mm1+mm2 for a quad is 20 interleaved
   N=512 streams over 2 lanes -> ~2.1 us warm, under the quad's DMA
   time (~3 us) -> the steady state is input-DMA-bound.
 - ph/po PSUM tiles hold a whole quad ([128,1024] = 2 banks), so each
   elementwise instruction covers TWO groups (1024 cols), halving the
   per-instruction overhead that throttled v3: per quad just 4 ops,
   balanced 2+2 over ACT and DVE (relu-A + cast-B on ACT, relu-B via
   tensor_scalar + cast-A on DVE).
 - hT is 3 persistent [65,1024] lane buffers whose row 64 is a
   constant 1 (written once) so W2's extra row folds b2 into mm2.
 - Input DMA: progressive regions; output: chunked flushes on the
   scalar HWDGE queue overlapping compute.  25 groups total (12800
   rows, only 300 rows of padding); the last "quad" has one group.
"""

import math

import numpy as np

N, E = 100000, 1600000
F_IN, HID, CLS, K = 500, 64, 64, 10
KC = 128                     # contraction chunk partitions (features padded 500->512)
HIDP = HID + 1               # extra constant-1 hidden row carries b2
N_CORES = 8
RPC = N // N_CORES           # real rows per core: 12500
GROUP = 512                  # rows per PSUM matmul group
NG_TOT = 25                  # groups per core (6 quads + 1 lone group)
RPAD = NG_TOT * GROUP        # padded rows per core: 12800
NQ = (NG_TOT + 3) // 4       # quad iterations (last may be partial)
N_HT = 6                     # hT lane-buffer ring (3 quads of WAR slack)
OUT_FLUSH = [2048, 4096, 6144, 8192, 10240, 12288, 12800]
# tapered input-DMA regions (rows), all on the sync HWDGE queue: a
# matmul waits on its whole region's completion, descriptors must stay
# >= 4KB for stream rate, and the last regions are small (their
# completion latency gates the tail).  Splitting regions across both
# HWDGE queues was tried and is WORSE: fair packet round-robin makes
# every region complete later.
XT_REGIONS = [(0, 512), (512, 1536), (2048, 2048), (4096, 2048),
              (6144, 2048), (8192, 2048), (10240, 1536), (11776, 1024)]
NAT_LOG_EXP_SET = 6          # act_info.json id of natural_log_exp_and_others

_CACHE: dict = {}


def _bernstein_monomial_coeffs(temp: np.ndarray) -> np.ndarray:
    """Exact monomial coefficients c_j of
    (1/2^K) sum_i C(K,i) TEMP[i] (I-A)^i (I+A)^{K-i}  in powers of A.

    Uses float64 on small integers (exactly representable), so for
    TEMP == 1 the j>=1 coefficients cancel to exactly 0.0.
    """
    TEMP = np.maximum(temp.astype(np.float64), 0.0)
    c = np.zeros(K + 1, dtype=np.float64)
    for i in range(K + 1):
        p1 = np.array([math.comb(i, j) * ((-1.0) ** j) for j in range(i + 1)])
        p2 = np.array([math.comb(K - i, j) * 1.0 for j in range(K - i + 1)])
        c += math.comb(K, i) * TEMP[i] * np.convolve(p1, p2)
    return c / (2.0 ** K)


def _host_reference(x, edge_index, W1, b1, W2, b2, temp):
    """Full-fidelity host fallback (general temp).  Never hit for the
    setup_inputs() distribution (temp == ones); kept for correctness."""
    h = np.maximum(x @ W1 + b1, 0.0) @ W2 + b2
    row, col = edge_index[0].astype(np.int64), edge_index[1].astype(np.int64)
    deg = np.bincount(row, minlength=N).astype(np.float32)
    dis = np.where(deg > 0, 1.0 / np.sqrt(np.where(deg > 0, deg, 1.0)), 0.0)
    w = (dis[row] * dis[col]).astype(np.float32)
    try:
        import scipy.sparse as sp

        A = sp.csr_matrix((w, (row, col)), shape=(N, N), dtype=np.float32)

        def Av(v):
            return A @ v
    except ImportError:
        order = np.argsort(row, kind="stable")
        rs, cs, ws = row[order], col[order], w[order]
        starts = np.searchsorted(rs, np.arange(N))

        def Av(v):
            contrib = ws[:, None] * v[cs]
            out = np.add.reduceat(
                np.concatenate([contrib, np.zeros((1, v.shape[1]), v.dtype)]),
                np.minimum(starts, len(rs)),
                axis=0,
            )[:N]
            out[np.diff(np.append(starts, len(rs))) == 0] = 0
            return out

    TEMP = np.maximum(temp, 0.0)
    tmp = [h]
    v = h
    for _ in range(K):
        v = v + Av(v)
        tmp.append(v)
    out = (math.comb(K, 0) / 2 ** K) * TEMP[0] * tmp[K]
    for i in range(K):
        v = tmp[K - i - 1]
        for _ in range(i + 1):
            v = v - Av(v)
        out = out + (math.comb(K, i + 1) / 2 ** K) * TEMP[i + 1] * v
    m = out.max(axis=1, keepdims=True)
    return (out - m - np.log(np.exp(out - m).sum(axis=1, keepdims=True))).astype(
        np.float32
    )


def _dedupe_act_table_loads(nc, mybir):
    """Rewrite every act-table load to NAT_LOG_EXP_SET (covers Relu,
    Identity) and drop all but the first load per block."""
    for blk in nc.main_func.blocks:
        seen = False
        keep = []
        for inst in blk.instructions:
            if isinstance(inst, mybir.InstLoadActFuncSet):
                inst.act_func_set_id = NAT_LOG_EXP_SET
                plain = (
                    not inst.sync_info
                    and not inst.has_wait()
                    and not inst.has_update()
                )
                if seen and plain:
                    continue
                seen = True
            keep.append(inst)
        if len(keep) != len(blk.instructions):
            del blk.instructions[:]
            for inst in keep:
                blk.instructions.append(inst)


def _build_nc():
    """Build + compile the per-core Bass module (cached)."""
    if "nc" in _CACHE:
        return _CACHE["nc"]

    import concourse.bass as bass
    import concourse.tile as tile
    from concourse import bacc, mybir

    f32 = mybir.dt.float32
    f16 = mybir.dt.float16
    f8 = mybir.dt.float8e3
    AF = mybir.ActivationFunctionType
    ALU = mybir.AluOpType

    nc = bacc.Bacc("TRN2", target_bir_lowering=False, debug=False)

    xt = nc.declare_dram_parameter("xt", [KC, 4 * RPAD], f8, isOutput=False)
    w1 = nc.declare_dram_parameter("w1", [KC, 4 * HID], f16, isOutput=False)
    b1 = nc.declare_dram_parameter("b1", [HID, 1], f32, isOutput=False)
    w2 = nc.declare_dram_parameter("w2", [HIDP, CLS], f16, isOutput=False)
    out = nc.declare_dram_parameter("out", [CLS, RPAD], f16, isOutput=True)

    def groups_of(q):
        return [g for g in range(4 * q, 4 * q + 4) if g < NG_TOT]

    with tile.TileContext(nc) as tc:
        with (
            tc.tile_pool(name="const", bufs=1) as constp,
            tc.tile_pool(name="outp", bufs=3) as op,
            tc.tile_pool(name="psum", bufs=2, space=bass.MemorySpace.PSUM) as pp,
            tc.tile_pool(name="psum3", bufs=2, space=bass.MemorySpace.PSUM) as pp3,
        ):
            # --- weights on the scalar HWDGE queue: concurrent with the
            # x stream on the sync queue (both rings are idle at start).
            # w1 is pre-baked host-side to one contiguous 512 B run per
            # partition (the naive rearrange was 512 x 128 B descriptors
            # and stalled the queue ~5 us).
            w1_sb = constp.tile([KC, 4, HID], f16)
            nc.scalar.dma_start(out=w1_sb[:], in_=w1[:])
            w2_sb = constp.tile([HIDP, CLS], f16)
            nc.scalar.dma_start(out=w2_sb[:], in_=w2[:])
            b1_sb = constp.tile([HID, 1], f32)
            nc.scalar.dma_start(out=b1_sb[:], in_=b1[:])

            # persistent hT lane buffers (2 groups each); row 64 is the
            # constant 1 that turns W2's b2 row into a bias add in mm2.
            ht_bufs = []
            for i in range(N_HT):
                hb = constp.tile([HIDP, 2 * GROUP], f16, name=f"htb{i}")
                ht_bufs.append(hb)

            # Warmup/pacing bridge.  The x stream supplies ~360 GB/s but
            # the column-tiled PE consumes ~490, so a PE that starts as
            # soon as region 0 lands keeps catching up to the stream in
            # 1-2 us bursts that re-throttle the HAM clock gate (half
            # clock) and pay the region-completion latency repeatedly.
            # Instead, concentrate ALL the deficit at the start: batches
            # of throwaway matmuls gated on successive regions keep the
            # PE continuously busy until ~3 regions are resident, after
            # which the real stream runs warm and uninterrupted to the
            # end.  The throwaway results land in ph0 windows that the
            # first real start=True matmuls overwrite.
            warm_in = constp.tile([KC, GROUP], f16)
            nc.vector.memset(warm_in[:], 0.0)
            for hb in ht_bufs:
                nc.vector.memset(hb[HID : HID + 1, :], 1.0)

            xt_all = constp.tile([KC, 4 * RPAD], f8)
            reg_of = {}
            for ri, (s, rows) in enumerate(XT_REGIONS):
                nc.sync.dma_start(
                    out=xt_all[:, 4 * s : 4 * s + 4 * rows],
                    in_=xt[:, 4 * s : 4 * s + 4 * rows],
                )
                for g in range(s // GROUP, (s + rows) // GROUP):
                    reg_of[g] = (s, rows)

            def xt_ap(g, ci):
                s, rows = reg_of[g]
                a = g * GROUP - s
                off = 4 * s + ci * rows + a
                return xt_all[:, off : off + GROUP]

            # Warmup: 8 throwaway matmuls rotating over ph0's four
            # windows (same-window WAW back-to-back would serialize at
            # full fill+drain latency) bridge the PE from t~8 until
            # region 0 lands; the first real start=True matmuls then
            # overwrite ph0.
            ph0 = pp3.tile([KC, 2 * GROUP], f32, tag="ph")
            phs = {0: ph0}
            for wi in range(8):
                rp = 64 * (wi % 2)
                cw = GROUP * ((wi // 2) % 2)
                nc.tensor.matmul(
                    ph0[rp : rp + 64, cw : cw + GROUP],
                    lhsT=warm_in[:, 0:64], rhs=warm_in[:],
                    start=True, stop=True, tile_position=(0, rp),
                )

            def win(tile_, g):
                """[64,512] window of a quad PSUM tile for group g."""
                rp = 64 * ((g % 4) // 2)
                cw = GROUP * (g % 2)
                return tile_[rp : rp + 64, cw : cw + GROUP]

            pos = {}
            ots = {}
            pend_flush = []

            def s0_mm1(q):
                """mm1 for the quad.  Early quads (the DMA-deficit
                window) issue lane-major/serial so the PE consumes at
                ~245 GB/s and trails the ~360 GB/s stream via its
                natural data deps, staying continuously busy (warm HAM,
                no catch-up bursts).  Later quads interleave lane A and
                lane B so both column tiles stream concurrently at 2x.
                """
                gs = groups_of(q)
                if q in phs:
                    ph = phs[q]
                else:
                    ph = pp3.tile([KC, 2 * GROUP], f32, tag="ph")
                    phs[q] = ph
                laneA = [g for g in gs if (g % 4) < 2]
                laneB = [g for g in gs if (g % 4) >= 2]
                order = []
                for ci in range(4):
                    for sub in range(max(len(laneA), len(laneB))):
                        for lane in (laneA, laneB):
                            if sub < len(lane):
                                order.append((lane[sub], ci))
                for g, ci in order:
                    cp = 64 * ((g % 4) // 2)
                    nc.tensor.matmul(
                        win(ph, g),
                        lhsT=w1_sb[:, ci, :], rhs=xt_ap(g, ci),
                        start=(ci == 0), stop=(ci == 3),
                        tile_position=(0, cp),
                    )

            def s1_mid(q):
                gs = groups_of(q)
                ph = phs.pop(q)
                nA = len([g for g in gs if (g % 4) < 2])
                nB = len(gs) - nA
                hbA = ht_bufs[(2 * q) % N_HT]
                hbB = ht_bufs[(2 * q + 1) % N_HT]
                # relu lane A on ACT, lane B on DVE (tensor_scalar)
                if nA:
                    nc.scalar.activation(
                        hbA[0:HID, 0 : nA * GROUP], ph[0:64, 0 : nA * GROUP],
                        AF.Relu, bias=b1_sb[:],
                    )
                if nB:
                    nc.vector.tensor_scalar(
                        hbB[0:HID, 0 : nB * GROUP], ph[64:128, 0 : nB * GROUP],
                        scalar1=b1_sb[:], scalar2=0.0,
                        op0=ALU.add, op1=ALU.max,
                    )
                po = pp.tile([KC, 2 * GROUP], f32, tag="po")
                pos[q] = po
                for sub in range(2):
                    for g in gs:
                        if g % 2 != sub:
                            continue
                        cp = 64 * ((g % 4) // 2)
                        hb = hbA if (g % 4) < 2 else hbB
                        cw = GROUP * (g % 2)
                        nc.tensor.matmul(
                            win(po, g), lhsT=w2_sb[:],
                            rhs=hb[:, cw : cw + GROUP],
                            start=True, stop=True, tile_position=(0, cp),
                        )

            def s2_fin(q):
                # issue the previous quad's flush now: its casts are
                # long done, so the DMA trigger does not wait and
                # cannot head-of-line block the scalar engine.
                while pend_flush:
                    ft, fbnd, ftile = pend_flush.pop(0)
                    nc.scalar.dma_start(out=out[:, ft:fbnd], in_=ftile[:])
                gs = groups_of(q)
                po = pos.pop(q)
                g_end = (gs[-1] + 1) * GROUP
                bnd_prev = 0
                for b in OUT_FLUSH:
                    if b >= g_end:
                        bnd = b
                        break
                    bnd_prev = b
                t = bnd_prev
                if t not in ots:
                    ot_new = op.tile([CLS, bnd - bnd_prev], f16, tag="ot")
                    ots[t] = ot_new
                nA = len([g for g in gs if (g % 4) < 2])
                nB = len(gs) - nA
                c0 = gs[0] * GROUP - t
                # cast lane A on DVE, lane B on ACT
                if nA:
                    nc.vector.tensor_copy(
                        ots[t][:, c0 : c0 + nA * GROUP], po[0:64, 0 : nA * GROUP]
                    )
                if nB:
                    nc.scalar.activation(
                        ots[t][:, c0 + nA * GROUP : c0 + (nA + nB) * GROUP],
                        po[64:128, 0 : nB * GROUP], AF.Copy,
                    )
                if g_end == bnd:
                    if bnd == OUT_FLUSH[-1]:
                        # final flush: nothing left on the scalar
                        # engine, issue immediately.
                        nc.scalar.dma_start(out=out[:, t:bnd], in_=ots.pop(t)[:])
                    else:
                        pend_flush.append((t, bnd, ots.pop(t)))

            for qq in range(NQ + 2):
                if qq < NQ:
                    s0_mm1(qq)
                if 1 <= qq <= NQ:
                    s1_mid(qq - 1)
                if 2 <= qq <= NQ + 1:
                    s2_fin(qq - 2)

    nc.compile()
    _dedupe_act_table_loads(nc, mybir)
    _CACHE["nc"] = nc
    return nc


def _bake_xt(x_rows: np.ndarray) -> np.ndarray:
    """[<=RPC, F_IN] row slice -> flat SBUF image [KC, 4*RPAD] fp8e3m4,
    features padded to 512 in 4 chunks of 128, laid out per DMA region
    as [partition][chunk][row-in-region]."""
    import ml_dtypes

    xp_ = np.zeros((4 * KC, RPAD), ml_dtypes.float8_e3m4)
    xp_[:F_IN, : x_rows.shape[0]] = x_rows.T.astype(ml_dtypes.float8_e3m4)
    parts = []
    for (s, rows) in XT_REGIONS:
        blk = xp_[:, s : s + rows].reshape(4, KC, rows).transpose(1, 0, 2)
        parts.append(blk.reshape(KC, 4 * rows))
    return np.ascontiguousarray(np.concatenate(parts, axis=1))


def kernel(**inputs: np.ndarray) -> np.ndarray:
    x = np.asarray(inputs["x"], dtype=np.float32)
    W1 = np.asarray(inputs["W1"], dtype=np.float32)
    b1 = np.asarray(inputs["b1"], dtype=np.float32)
    W2 = np.asarray(inputs["W2"], dtype=np.float32)
    b2 = np.asarray(inputs["b2"], dtype=np.float32)
    temp = np.asarray(inputs["temp"], dtype=np.float32)

    c = _bernstein_monomial_coeffs(temp)
    if np.any(c[1:] != 0.0) or c[0] != 1.0:
        # General temp: graph propagation actually matters — host fallback.
        return _host_reference(
            x, np.asarray(inputs["edge_index"]), W1, b1, W2, b2, temp
        )

    from concourse.bass_utils import run_bass_kernel_spmd

    nc = _build_nc()
    W1f = np.zeros((4 * KC, HID), np.float16)
    W1f[:F_IN, :] = W1.astype(np.float16)
    # [KC, 4*HID]: chunk-major per partition, matching w1_sb [KC,4,HID]
    W1p = np.ascontiguousarray(
        W1f.reshape(4, KC, HID).transpose(1, 0, 2).reshape(KC, 4 * HID)
    )
    b1p = np.ascontiguousarray(b1.reshape(HID, 1))
    # W2' = [W2 ; b2]: the constant-1 hT row turns row 64 into + b2
    W2p = np.ascontiguousarray(
        np.concatenate([W2, b2.reshape(1, CLS)], axis=0)
    ).astype(np.float16)

    in_maps = []
    for cix in range(N_CORES):
        in_maps.append(
            {
                "xt": _bake_xt(x[cix * RPC : (cix + 1) * RPC]),
                "w1": W1p,
                "b1": b1p,
                "w2": W2p,
            }
        )

    res = run_bass_kernel_spmd(nc, in_maps, list(range(N_CORES)))
    po = np.empty((N, CLS), np.float32)
    for cix in range(N_CORES):
        o = res.results[cix]["out"]
        po[cix * RPC : (cix + 1) * RPC] = o[:, :RPC].T
    # softmax epilogue (cheap, bandwidth-free on device): log_softmax(po)
    m = po.max(axis=1, keepdims=True)
    po -= m + np.log(np.exp(po - m).sum(axis=1, keepdims=True))
    return po
